# revision 18
# baseline (speedup 1.0000x reference)
"""Trainium2 Bass kernel for nn_ComplexityAttention (GQA attention block).

Computation (B=1, S=2048, HID=2048, 16 Q heads / 4 KV heads, D=128):
  q/k/v = x @ W^T + mu @ Wm^T           (fused mu-guided projections)
  per-head RMSNorm on q, k; RoPE; causal GQA attention; out @ wo^T.

Sharding: tensor-parallel over heads across 8 NeuronCores. Core c owns
Q heads {2c, 2c+1} and KV head c//2 (KV work duplicated per core pair).
Each core produces a partial output (its heads' slice of wo applied),
host sums the 8 partials.

Device-side layout strategy:
  - Host pre-transposes x/mu to [HID, S] and weights to [HID, out] so all
    matmuls contract over the partition dim with no on-device transposes
    for the projections.
  - Projections computed in [s, d] tiles (one PSUM bank holds q0|q1|k|v),
    RMSNorm+RoPE done with per-partition scalars + free-dim shifts
    (fused scalar_tensor_tensor ops), then Q/K PE-transposed to [d, s]
    for attention.
  - Scores computed transposed: S^T[kv, q] = K^T.T @ Q^T. Softmax without
    max-subtraction (scores bounded by +/-sqrt(128) after RMSNorm, exp is
    safe in fp32); denominator via ones-vector matmul; causal masking via
    4 static multiplicative masks on the diagonal tiles.
  - PV: out^T[d, q] = V[kv, d].T @ expS^T[kv, q] accumulated over kv chunks.
  - Output projection from out^T directly; partial written as [o, s] fp32.

All matmul inputs are bf16 (fp32 PSUM accumulation); statistics in fp32.
"""

import sys

for _p in ("/opt/trn_rl_repo", "/root/.axon_site/_ro/trn_rl_repo"):
    if _p not in sys.path:
        sys.path.insert(0, _p)

import numpy as np
import ml_dtypes

import concourse.bass as bass
import concourse.bacc as bacc
import concourse.mybir as mybir
import concourse.tile as tile
from concourse.bass_utils import run_bass_kernel_spmd
from concourse.masks import make_identity

# Problem constants (hardcoded per contract)
B, S, HID = 1, 2048, 2048
NUM_HEADS, NUM_KV_HEADS, HEAD_DIM = 16, 4, 128
ROPE_THETA = 10000.0
EPS = 1e-6
N_CORES = 8

P = 128
KC = HID // P            # 16 contraction chunks
SC = S // P              # 16 sequence chunks of 128
QCH = 512                # attention q-chunk (one PSUM bank)
NQC = S // QCH           # 4
NPASS = 8                # projection passes (2 s-chunks each)
SCP = SC // NPASS        # s-chunks per pass = 2
QK_SCALE = 1.0 / float(np.sqrt(HEAD_DIM))

BF16 = mybir.dt.bfloat16
F32 = mybir.dt.float32
NP_BF16 = ml_dtypes.bfloat16

_PROGRAM = None  # cached (nc, input names)


def _build_program():
    """Build the per-core Bass/Tile program (identical on all 8 cores)."""
    AF = mybir.ActivationFunctionType
    OP = mybir.AluOpType

    nc = bacc.Bacc(trn_type="TRN2", debug=False)

    # ---- DRAM I/O ----
    xT = nc.dram_tensor("xT", [KC, P, S], BF16, kind="ExternalInput")
    muT = nc.dram_tensor("muT", [KC, P, S], BF16, kind="ExternalInput")
    # packed projection weights: [q0 | q1 | k | v] columns, transposed to [HID, 512]
    w_all = nc.dram_tensor("w_all", [KC, P, 512], BF16, kind="ExternalInput")
    wm_all = nc.dram_tensor("wm_all", [KC, P, 512], BF16, kind="ExternalInput")
    woT = nc.dram_tensor("woT", [2, P, HID], BF16, kind="ExternalInput")
    cosq = nc.dram_tensor("cosq", [SC, P, HEAD_DIM], F32, kind="ExternalInput")
    sinq = nc.dram_tensor("sinq", [SC, P, HEAD_DIM], F32, kind="ExternalInput")
    cosk = nc.dram_tensor("cosk", [SC, P, HEAD_DIM], F32, kind="ExternalInput")
    sink = nc.dram_tensor("sink", [SC, P, HEAD_DIM], F32, kind="ExternalInput")
    out_d = nc.dram_tensor("out", [KC, P, S], F32, kind="ExternalOutput")

    with tile.TileContext(nc) as tc:
        with (
            tc.tile_pool(name="persist", bufs=1) as persist,
            tc.tile_pool(name="stream", bufs=8) as stream,
            tc.tile_pool(name="tmp", bufs=4) as tmp,
            tc.tile_pool(name="small", bufs=6) as small,
            tc.tile_pool(name="expp", bufs=3) as expp,
            tc.tile_pool(name="ostage", bufs=3) as ostage,
            tc.tile_pool(name="ps_big", bufs=6, space="PSUM") as ps_big,
            tc.tile_pool(name="ps_scr", bufs=2, space="PSUM") as ps_scr,
        ):
            # ---- persistent SBUF tensors ----
            w_sb = persist.tile([P, KC, 512], BF16, name="w_sb")
            wm_sb = persist.tile([P, KC, 512], BF16, name="wm_sb")
            wo_sb = persist.tile([P, 2, HID], BF16, name="wo_sb")
            cq_sb = persist.tile([P, SC, HEAD_DIM], F32, name="cq_sb")
            sq_sb = persist.tile([P, SC, HEAD_DIM], F32, name="sq_sb")
            ck_sb = persist.tile([P, SC, HEAD_DIM], F32, name="ck_sb")
            sk_sb = persist.tile([P, SC, HEAD_DIM], F32, name="sk_sb")
            qt_sb = [
                persist.tile([P, S], BF16, name=f"qt{h}_sb") for h in range(2)
            ]
            kt_sb = persist.tile([P, S], BF16, name="kt_sb")
            v_sb = persist.tile([P, SC, HEAD_DIM], BF16, name="v_sb")
            attn_sb = [
                persist.tile([P, S], BF16, name=f"attn{c}_sb") for c in range(2)
            ]
            ident = persist.tile([P, P], BF16, name="ident")
            ones_sb = persist.tile([P, 1], BF16, name="ones_sb")
            eps_sb = persist.tile([P, 1], F32, name="eps_sb")
            masks = [
                persist.tile([P, QCH], BF16, name=f"mask{r}") for r in range(4)
            ]

            nc.sync.dma_start(w_sb[:], w_all.ap().rearrange("k p n -> p k n"))
            nc.sync.dma_start(wm_sb[:], wm_all.ap().rearrange("k p n -> p k n"))
            nc.sync.dma_start(wo_sb[:], woT.ap().rearrange("c p n -> p c n"))
            nc.sync.dma_start(cq_sb[:], cosq.ap().rearrange("s p d -> p s d"))
            nc.sync.dma_start(sq_sb[:], sinq.ap().rearrange("s p d -> p s d"))
            nc.sync.dma_start(ck_sb[:], cosk.ap().rearrange("s p d -> p s d"))
            nc.sync.dma_start(sk_sb[:], sink.ap().rearrange("s p d -> p s d"))

            make_identity(nc, ident[:])
            nc.gpsimd.memset(ones_sb[:], 1.0)
            nc.gpsimd.memset(eps_sb[:], EPS)
            for r in range(4):
                # keep 1.0 where (q_local - kv_local - 128*r) >= 0, else 0
                nc.gpsimd.memset(masks[r][:], 1.0)
                nc.gpsimd.affine_select(
                    out=masks[r][:],
                    in_=masks[r][:],
                    compare_op=mybir.AluOpType.is_ge,
                    fill=0.0,
                    base=-(P * r),
                    pattern=[[1, QCH]],
                    channel_multiplier=-1,
                )

            # head offsets inside the packed 512-wide projection output
            # (q0, q1, k occupy 0:128, 128:256, 256:384 and get norm+rope;
            #  v occupies 384:512)
            norm_specs = [
                (0, cq_sb, sq_sb, qt_sb[0]),
                (1, cq_sb, sq_sb, qt_sb[1]),
                (2, ck_sb, sk_sb, kt_sb),
            ]

            def do_attention_chunk(qc):
                """Attention + output projection for q columns [qc*512, ...)."""
                jpq = QCH // P  # kv chunks per q chunk
                jmax = jpq * qc + (jpq - 1)
                for h in range(2):
                    out_ps = ps_big.tile([P, QCH], F32, tag="big", name="out_ps")
                    den_ps = ps_scr.tile([1, QCH], F32, tag="scr", name="den_ps")
                    for j in range(jmax + 1):
                        s_ps = ps_big.tile([P, QCH], F32, tag="big", name="s_ps")
                        nc.tensor.matmul(
                            s_ps[:],
                            kt_sb[:, j * P : (j + 1) * P],
                            qt_sb[h][:, qc * QCH : (qc + 1) * QCH],
                            start=True,
                            stop=True,
                        )
                        e = expp.tile([P, QCH], BF16, tag="e", name="e")
                        nc.scalar.activation(e[:], s_ps[:], AF.Exp, scale=QK_SCALE)
                        r = j - jpq * qc
                        if r >= 0:
                            nc.vector.tensor_mul(e[:], e[:], masks[r][:])
                        nc.tensor.matmul(
                            out_ps[:],
                            v_sb[:, j, :],
                            e[:],
                            start=(j == 0),
                            stop=(j == jmax),
                        )
                        nc.tensor.matmul(
                            den_ps[:],
                            ones_sb[:],
                            e[:],
                            start=(j == 0),
                            stop=(j == jmax),
                        )
                    rd = small.tile([1, QCH], F32, tag="rd", name="rd")
                    nc.vector.reciprocal(rd[:], den_ps[:])
                    rdb = tmp.tile([P, QCH], F32, tag="rdb", name="rdb")
                    nc.gpsimd.partition_broadcast(rdb[:], rd[:])
                    nc.vector.tensor_mul(
                        attn_sb[h][:, qc * QCH : (qc + 1) * QCH], out_ps[:], rdb[:]
                    )
                # output projection for this q chunk: out_pT[o, q] partial
                for oc in range(KC):
                    o_ps = ps_scr.tile([P, QCH], F32, tag="scr", name="o_ps")
                    for c in range(2):
                        nc.tensor.matmul(
                            o_ps[:],
                            wo_sb[:, c, oc * P : (oc + 1) * P],
                            attn_sb[c][:, qc * QCH : (qc + 1) * QCH],
                            start=(c == 0),
                            stop=(c == 1),
                        )
                    ob = ostage.tile([P, QCH], F32, tag="ob", name="ob")
                    nc.scalar.copy(ob[:], o_ps[:])
                    nc.sync.dma_start(
                        out_d.ap()[oc, :, qc * QCH : (qc + 1) * QCH], ob[:]
                    )

            for p in range(NPASS):
                col0 = p * SCP * P  # first s column of this pass (512 wide)
                psums = [
                    ps_big.tile([P, 512], F32, tag="big", name=f"proj{p}_{i}")
                    for i in range(SCP)
                ]
                # x @ W^T contributions
                for kc in range(KC):
                    xt = stream.tile([P, SCP * P], BF16, tag="xt", name="xt")
                    nc.sync.dma_start(xt[:], xT.ap()[kc, :, col0 : col0 + SCP * P])
                    for i in range(SCP):
                        nc.tensor.matmul(
                            psums[i][:],
                            xt[:, i * P : (i + 1) * P],
                            w_sb[:, kc, :],
                            start=(kc == 0),
                            stop=False,
                        )
                # mu @ Wm^T contributions
                for kc in range(KC):
                    mt = stream.tile([P, SCP * P], BF16, tag="mt", name="mt")
                    nc.sync.dma_start(mt[:], muT.ap()[kc, :, col0 : col0 + SCP * P])
                    for i in range(SCP):
                        nc.tensor.matmul(
                            psums[i][:],
                            mt[:, i * P : (i + 1) * P],
                            wm_sb[:, kc, :],
                            start=False,
                            stop=(kc == KC - 1),
                        )
                # RMSNorm + RoPE + transpose to [d, s]; V copy
                for i in range(SCP):
                    sc = p * SCP + i
                    ps = psums[i]
                    for hidx, c_sb, s_sb, dst in norm_specs:
                        off = hidx * P
                        sqv = tmp.tile([P, HEAD_DIM], F32, tag="sqv", name="sqv")
                        var = small.tile([P, 1], F32, tag="var", name="var")
                        nc.scalar.activation(
                            sqv[:], ps[:, off : off + P], AF.Square, accum_out=var[:]
                        )
                        std = small.tile([P, 1], F32, tag="std", name="std")
                        # std = sqrt(sum(q^2)/D + eps)
                        nc.scalar.activation(
                            std[:], var[:], AF.Sqrt, scale=1.0 / HEAD_DIM, bias=eps_sb[:]
                        )
                        rstd = small.tile([P, 1], F32, tag="rstd", name="rstd")
                        nc.vector.reciprocal(rstd[:], std[:])
                        t1 = tmp.tile([P, HEAD_DIM], F32, tag="t1", name="t1")
                        nc.vector.scalar_tensor_tensor(
                            t1[:],
                            ps[:, off : off + P],
                            rstd[:],
                            c_sb[:, sc, :],
                            op0=OP.mult,
                            op1=OP.mult,
                        )
                        t2 = tmp.tile([P, HEAD_DIM], F32, tag="t2", name="t2")
                        nc.vector.scalar_tensor_tensor(
                            t2[:, 0:64],
                            ps[:, off + 64 : off + P],
                            rstd[:],
                            s_sb[:, sc, 0:64],
                            op0=OP.mult,
                            op1=OP.mult,
                        )
                        nc.vector.scalar_tensor_tensor(
                            t2[:, 64:P],
                            ps[:, off : off + 64],
                            rstd[:],
                            s_sb[:, sc, 64:P],
                            op0=OP.mult,
                            op1=OP.mult,
                        )
                        qsd = tmp.tile([P, HEAD_DIM], BF16, tag="qsd", name="qsd")
                        nc.vector.tensor_add(qsd[:], t1[:], t2[:])
                        tr = ps_scr.tile([P, P], BF16, tag="scr", name="tr")
                        nc.tensor.transpose(tr[:], qsd[:], ident[:])
                        nc.vector.tensor_copy(
                            dst[:, sc * P : (sc + 1) * P], tr[:]
                        )
                    # V: plain copy (cast) into [s, d] layout
                    nc.scalar.copy(v_sb[:, sc, :], ps[:, 384:512])
            # attention + output projection, after all projections
            # (keeps ACT on one table set per phase: sqrt/square first, exp after)
            for qc in range(NQC):
                do_attention_chunk(qc)

    nc.compile()
    return nc


def _get_program():
    global _PROGRAM
    if _PROGRAM is None:
        _PROGRAM = _build_program()
    return _PROGRAM


def _host_prepare(inputs):
    """Shard + lay out inputs for the 8 cores."""
    hs = np.asarray(inputs["hidden_states"], dtype=np.float32).reshape(S, HID)
    mu = np.asarray(inputs["mu_prev"], dtype=np.float32).reshape(S, HID)
    wq = np.asarray(inputs["wq"], dtype=np.float32)
    wk = np.asarray(inputs["wk"], dtype=np.float32)
    wv = np.asarray(inputs["wv"], dtype=np.float32)
    wo = np.asarray(inputs["wo"], dtype=np.float32)
    wmq = np.asarray(inputs["wmq"], dtype=np.float32)
    wmk = np.asarray(inputs["wmk"], dtype=np.float32)
    wmv = np.asarray(inputs["wmv"], dtype=np.float32)
    qw = np.asarray(inputs["q_norm_w"], dtype=np.float32)
    kw = np.asarray(inputs["k_norm_w"], dtype=np.float32)

    xT = np.ascontiguousarray(hs.T).astype(NP_BF16).reshape(KC, P, S)
    muT = np.ascontiguousarray(mu.T).astype(NP_BF16).reshape(KC, P, S)

    # RoPE tables in [s, d] layout with rotate-half sign and norm weight baked in
    inv = 1.0 / (ROPE_THETA ** (np.arange(0, HEAD_DIM, 2, dtype=np.float32) / HEAD_DIM))
    ang = np.arange(S, dtype=np.float32)[:, None] * inv[None, :]  # [S, 64]
    emb = np.concatenate([ang, ang], axis=-1)  # [S, 128]
    cos_e = np.cos(emb)
    sin_e = np.sin(emb)
    sin_s = np.concatenate([-sin_e[:, :64], sin_e[:, 64:]], axis=-1)

    def tables(w):
        w_shift = np.concatenate([w[64:], w[:64]])
        cos_t = (cos_e * w[None, :]).astype(np.float32).reshape(SC, P, HEAD_DIM)
        sin_t = (sin_s * w_shift[None, :]).astype(np.float32).reshape(SC, P, HEAD_DIM)
        return np.ascontiguousarray(cos_t), np.ascontiguousarray(sin_t)

    cq, sq = tables(qw)
    ck, sk = tables(kw)

    in_maps = []
    for c in range(N_CORES):
        g = c // 2
        wq_s = wq[256 * c : 256 * (c + 1)]      # [256, HID]
        wmq_s = wmq[256 * c : 256 * (c + 1)]
        wk_s = wk[P * g : P * (g + 1)]          # [128, HID]
        wmk_s = wmk[P * g : P * (g + 1)]
        wv_s = wv[P * g : P * (g + 1)]
        wmv_s = wmv[P * g : P * (g + 1)]
        w_all = np.concatenate([wq_s.T, wk_s.T, wv_s.T], axis=1)     # [HID, 512]
        wm_all = np.concatenate([wmq_s.T, wmk_s.T, wmv_s.T], axis=1)
        woT_c = wo[:, 256 * c : 256 * (c + 1)].T                     # [256, HID]
        in_maps.append(
            {
                "xT": xT,
                "muT": muT,
                "w_all": np.ascontiguousarray(w_all).astype(NP_BF16).reshape(KC, P, 512),
                "wm_all": np.ascontiguousarray(wm_all).astype(NP_BF16).reshape(KC, P, 512),
                "woT": np.ascontiguousarray(woT_c).astype(NP_BF16).reshape(2, P, HID),
                "cosq": cq,
                "sinq": sq,
                "cosk": ck,
                "sink": sk,
            }
        )
    return in_maps


def run(inputs, trace=False):
    """Run the SPMD kernel; returns (full_output, exec_time_ns_or_None)."""
    nc = _get_program()
    in_maps = _host_prepare(inputs)
    res = run_bass_kernel_spmd(
        nc, in_maps, core_ids=list(range(N_CORES)), trace=trace
    )
    total = np.zeros((HID, S), dtype=np.float32)
    for c in range(N_CORES):
        total += res.results[c]["out"].reshape(HID, S)
    out = np.ascontiguousarray(total.T).reshape(B, S, HID).astype(np.float32)
    return out, res.exec_time_ns


def kernel(**inputs) -> np.ndarray:
    out, _ = run(inputs, trace=False)
    return out
